# revision 12
# baseline (speedup 1.0000x reference)
"""DistMult decoder kernel for Trainium2 (Bass/Tile), 8-core data-parallel.

Computes sigmoid(einsum('nd,d,nd->n', row, rel, col)) for N=500000, D=256.

Sharding: rows split evenly across 8 cores (62500 each). The selected
relation vector rel = relations[relation_index] is broadcast to [128, 256]
on host (tiny) and replicated to every core.

Per-core layout: the 62500-row shard is viewed as [128, 488, 256]
(n = p*488 + j) plus a 36-row tail, so every DMA moves per-partition
contiguous spans (16 KB for CHUNK=16). Engines:
  - HWDGE (sync + scalar rings): input loads
  - GPSIMD: prod = row * col   (elementwise)
  - DVE: tensor_tensor_reduce: scores[p, j] = sum_d(prod * rel)
  - ACT: sigmoid
"""

import numpy as np

import concourse.bass as bass
import concourse.mybir as mybir
from concourse import tile
from concourse.bass_utils import run_bass_kernel_spmd

N = 500000
D = 256
N_CORES = 8
N_SHARD = N // N_CORES  # 62500
P = 128
J = N_SHARD // P        # 488
MAIN = P * J            # 62464
TAIL = N_SHARD - MAIN   # 36
CHUNK = 16              # j-columns per DMA chunk (16 KB contiguous/partition)

F32 = mybir.dt.float32


def build_program(n_shard: int = N_SHARD, chunk: int = CHUNK, bufs: int = 3) -> bass.Bass:
    p = P
    j_cols = n_shard // p
    main = p * j_cols
    tail = n_shard - main

    nc = bass.Bass()
    row = nc.declare_dram_parameter("row", [n_shard, D], F32, isOutput=False)
    col = nc.declare_dram_parameter("col", [n_shard, D], F32, isOutput=False)
    relb = nc.declare_dram_parameter("relb", [p, D], F32, isOutput=False)
    out = nc.declare_dram_parameter("out", [n_shard], F32, isOutput=True)

    row_m = row[0:main, :].rearrange("(p j) d -> p j d", p=p)
    col_m = col[0:main, :].rearrange("(p j) d -> p j d", p=p)
    out_m = out[0:main].rearrange("(p j) -> p j", p=p)

    mult = mybir.AluOpType.mult
    add = mybir.AluOpType.add
    sig = mybir.ActivationFunctionType.Sigmoid

    # chunk schedule over the main part: sizes per chunk
    sizes = []
    j0 = 0
    while j0 < j_cols:
        sizes.append(min(chunk, j_cols - j0))
        j0 += chunk
    n_chunks = len(sizes)
    total_units = n_chunks + (1 if tail else 0)

    with (
        nc.sbuf_tensor([p, D], F32) as rel_sb,
        nc.sbuf_tensor([p, j_cols + 1], F32) as scores,
        nc.sbuf_tensor([p, bufs * chunk * D], F32) as rt_buf,
        nc.sbuf_tensor([p, bufs * chunk * D], F32) as ct_buf,
        nc.sbuf_tensor([p, D], F32) as rt_t,
        nc.sbuf_tensor([p, D], F32) as ct_t,
        nc.semaphore("rel_sem") as rel_sem,
        nc.semaphore("slot_sem0") as slot_sem0,
        nc.semaphore("slot_sem1") as slot_sem1,
        nc.semaphore("slot_sem2") as slot_sem2,
        nc.semaphore("tail_sem") as tail_sem,
        nc.semaphore("ve_sem") as ve_sem,
        nc.semaphore("act_sem") as act_sem,
        nc.semaphore("store_sem") as store_sem,
        nc.Block() as block,
    ):
        assert bufs == 3
        slot_sems = [slot_sem0, slot_sem1, slot_sem2]

        def rt_slot(c):
            b = c % bufs
            return rt_buf[:, b * chunk * D : (b + 1) * chunk * D]

        def ct_slot(c):
            b = c % bufs
            return ct_buf[:, b * chunk * D : (b + 1) * chunk * D]

        # ve_sem increment bookkeeping: every DVE instruction incs ve_sem by 1
        # (TT + k TTRs per chunk; TT_t + TTR_t for the tail). A wait for the
        # cumulative total of all incs issued so far guarantees completion of
        # every one of them, even with unordered retirement.
        cum = []
        tot = 0
        for k in sizes:
            tot += 1 + k
            cum.append(tot)
        cum_total = tot + (2 if tail else 0)

        @block.sync
        def _(sync):
            sync.dma_start(rel_sb[:], relb[:]).then_inc(rel_sem, 16)
            j0 = 0
            for c, k in enumerate(sizes):
                if c >= bufs:
                    # slot reuse: chunk c-bufs must be fully consumed
                    sync.wait_ge(ve_sem, cum[c - bufs])
                sem = slot_sems[c % bufs]
                sync.dma_start(
                    rt_slot(c)[:, 0 : k * D], row_m[:, j0 : j0 + k, :]
                ).then_inc(sem, 16)
                sync.dma_start(
                    ct_slot(c)[:, 0 : k * D], col_m[:, j0 : j0 + k, :]
                ).then_inc(sem, 16)
                j0 += k
            if tail:
                sync.dma_start(rt_t[0:tail, :], row[main:n_shard, :]).then_inc(
                    tail_sem, 16
                )
                sync.dma_start(ct_t[0:tail, :], col[main:n_shard, :]).then_inc(
                    tail_sem, 16
                )
            # store phase
            sync.wait_ge(act_sem, 2 if tail else 1)
            sync.dma_start(out_m, scores[:, 0:j_cols]).then_inc(store_sem, 16)
            if tail:
                sync.dma_start(
                    out[main:n_shard].rearrange("(p j) -> p j", j=1),
                    scores[0:tail, j_cols : j_cols + 1],
                ).then_inc(store_sem, 16)
            sync.wait_ge(store_sem, 32 if tail else 16)

        @block.vector
        def _(vector):
            vector.wait_ge(rel_sem, 16)
            j0 = 0
            for c, k in enumerate(sizes):
                vector.wait_ge(slot_sems[c % bufs], 32 * (c // bufs + 1))
                ct_v = ct_slot(c)[:, 0 : k * D].rearrange("p (k d) -> p k d", d=D)
                vector.tensor_tensor(
                    out=ct_v,
                    in0=ct_v,
                    in1=rel_sb[:].unsqueeze(1).broadcast_to([p, k, D]),
                    op=mult,
                ).then_inc(ve_sem, 1)
                # TT of this chunk (and everything before it) must be complete
                # before the TTRs read ct: wait for all incs issued so far.
                prior = (cum[c - 1] if c else 0) + 1
                vector.wait_ge(ve_sem, prior)
                for jj in range(k):
                    sl = ct_slot(c)[:, jj * D : (jj + 1) * D]
                    vector.scalar_tensor_tensor(
                        out=sl,
                        in0=sl,
                        scalar=1.0,
                        in1=rt_slot(c)[:, jj * D : (jj + 1) * D],
                        op0=mult,
                        op1=mult,
                        accum_out=scores[:, j0 + jj : j0 + jj + 1],
                    ).then_inc(ve_sem, 1)
                j0 += k
            if tail:
                vector.wait_ge(tail_sem, 32)
                vector.tensor_tensor(
                    out=ct_t[0:tail, :],
                    in0=ct_t[0:tail, :],
                    in1=rel_sb[0:tail, :],
                    op=mult,
                ).then_inc(ve_sem, 1)
                vector.wait_ge(ve_sem, cum[-1] + 1)
                vector.scalar_tensor_tensor(
                    out=ct_t[0:tail, :],
                    in0=ct_t[0:tail, :],
                    scalar=1.0,
                    in1=rt_t[0:tail, :],
                    op0=mult,
                    op1=mult,
                    accum_out=scores[0:tail, j_cols : j_cols + 1],
                ).then_inc(ve_sem, 1)

        @block.scalar
        def _(scalar):
            scalar.wait_ge(ve_sem, cum_total)
            scalar.activation(
                out=scores[:, 0:j_cols], in_=scores[:, 0:j_cols], func=sig
            ).then_inc(act_sem, 1)
            if tail:
                scalar.activation(
                    out=scores[0:tail, j_cols : j_cols + 1],
                    in_=scores[0:tail, j_cols : j_cols + 1],
                    func=sig,
                ).then_inc(act_sem, 1)

    return nc


_PROGRAM = None


def _get_program() -> bass.Bass:
    global _PROGRAM
    if _PROGRAM is None:
        _PROGRAM = build_program()
    return _PROGRAM


def _run(inputs_row, inputs_col, relations, relation_index, **spmd_kwargs):
    inputs_row = np.ascontiguousarray(np.asarray(inputs_row, dtype=np.float32))
    inputs_col = np.ascontiguousarray(np.asarray(inputs_col, dtype=np.float32))
    relations = np.asarray(relations, dtype=np.float32)
    idx = int(relation_index)

    rel = relations[idx]
    relb = np.ascontiguousarray(np.broadcast_to(rel, (P, D)))

    in_maps = []
    for c in range(N_CORES):
        sl = slice(c * N_SHARD, (c + 1) * N_SHARD)
        in_maps.append(
            {
                "row": inputs_row[sl],
                "col": inputs_col[sl],
                "relb": relb,
            }
        )

    nc = _get_program()
    return run_bass_kernel_spmd(nc, in_maps, list(range(N_CORES)), **spmd_kwargs)


def kernel(inputs_row, inputs_col, relations, relation_index):
    results = _run(inputs_row, inputs_col, relations, relation_index).results
    out = np.concatenate([results[c]["out"] for c in range(N_CORES)])
    return out.astype(np.float32, copy=False)


if __name__ == "__main__":
    rng = np.random.default_rng(0)
    inputs = {
        "inputs_row": rng.standard_normal((N, D), dtype=np.float32),
        "inputs_col": rng.standard_normal((N, D), dtype=np.float32),
        "relations": rng.standard_normal((8, D), dtype=np.float32),
        "relation_index": 3,
    }
    got = kernel(**inputs)
    rel = inputs["relations"][3]
    want = 1.0 / (
        1.0
        + np.exp(
            -np.einsum(
                "nd,d,nd->n", inputs["inputs_row"], rel, inputs["inputs_col"]
            )
        )
    )
    err = np.abs(got - want).max()
    print("max abs err:", err)
